# revision 1
# baseline (speedup 1.0000x reference)
"""Trainium2 Bass kernel for multi-head self-attention with RoPE.

Problem: y = MHSA(x) with
    qkv = x @ W_qkv  -> (B,S,3,H,hd) -> per-head q,k,v
    q,k = rope(q), rope(k)   (interleaved-pair rotary)
    out  = softmax(q k^T / sqrt(hd)) v
    y    = concat_heads(out) @ W_out
B=2, S=2048, E=2048, H=16, hd=128.

Sharding: 8 cores; core c handles batch b=c//4 and 4 heads h0=4*(c%4)..h0+3
(tensor-parallel over heads, data-parallel over batch). Each core computes a
partial output  O_part = attn_heads @ W_out[rows of its heads]  and the host
sums the 4 partials per batch.

Device-side layout strategy (all contractions need the reduced dim on SBUF
partitions):
  - host passes x TRANSPOSED (xT: E x S) so phase-1 needs no on-chip transpose
  - q,k are produced TRANSPOSED per head (qT/kT: [d=128, S]) via
      out = W_slice.T @ xT      (lhsT = W natural, rhs = xT natural)
    with the head dim PERMUTED even-dims-first (host permutes W columns) so
    RoPE needs no strided partition access.
  - v is produced NATURAL ([S, d]) via out = xT.T @ Wv.
  - scores are computed TRANSPOSED (scoresT[j,i] = k_j . q_i) via
      matmul(lhsT=kT, rhs=qT)
    softmax denominator = ones-matmul over partitions; exp stored fp16.
  - out2T[d,i] = sum_j v[j,d] expT[j,i] via matmul(lhsT=v, rhs=expT),
    normalized by 1/denom broadcast across partitions (gpsimd).
  - O_part[i,e] = sum_h out2T_h.T @ Wout_h via matmul(lhsT=out2T, rhs=Wout).
All big matmuls run as fp32r (FP22, full PE rate at free dim >= 256).
"""

import os
import math
import functools
from contextlib import ExitStack

import numpy as np

B, S, E = 2, 2048, 2048
A, H = 2048, 16
HD = A // H                     # 128
HPC = 4                         # heads per core
N_CORES = 8
THETA = 10000.0
SCALE = 1.0 / math.sqrt(HD)

LAST_RESULTS = None             # BassKernelResults of the last kernel() call


# --------------------------------------------------------------------------
# Bass program builder (parameterized so a small config can run in CoreSim)
# --------------------------------------------------------------------------
def build_bass(s=S, e=E, hpc=HPC, enable_asserts=False):
    import concourse.bass as bass
    import concourse.mybir as mybir
    import concourse.tile as tile
    from concourse import bacc

    f32 = mybir.dt.float32
    f32r = mybir.dt.float32r
    f16 = mybir.dt.float16
    Exp = mybir.ActivationFunctionType.Exp

    ES = e // 128               # e-subtiles (contraction)
    SC = 256                    # s-chunk width in projection phase
    NCH = s // SC               # projection chunks
    QKT = 2 * hpc               # packed qk c-tiles (q pairs then k pairs)
    VW = hpc * HD               # v width (=512 for hpc=4)
    NJT = s // 128              # j tiles (keys)
    IHS = min(1024, s)          # i-unit size
    IU = s // IHS               # i-units per head
    NCK = min(512, IHS)         # matmul free-dim chunk (one PSUM bank fp32)
    NIC = IHS // NCK

    nc = bacc.Bacc(
        "TRN2",
        target_bir_lowering=False,
        debug=False,
        enable_asserts=enable_asserts,
        num_devices=N_CORES,
    )

    xT = nc.dram_tensor("xT", (e, s), f16, kind="ExternalInput").ap()
    Wp = nc.dram_tensor("Wp", (e, QKT * 128 + VW), f16, kind="ExternalInput").ap()
    WoS = nc.dram_tensor("WoS", (VW, e), f32, kind="ExternalInput").ap()
    cosP = nc.dram_tensor("cosP", (128, s), f16, kind="ExternalInput").ap()
    sinP = nc.dram_tensor("sinP", (128, s), f16, kind="ExternalInput").ap()
    O = nc.dram_tensor("O_part", (s, e), f32, kind="ExternalOutput").ap()

    with tile.TileContext(nc) as tc, ExitStack() as octx:
        qkpool = octx.enter_context(tc.tile_pool(name="qkrot", bufs=1))
        qkrot = qkpool.tile([128, QKT, s], f16, tag="qkrot")    # rotated qT/kT
        vpool = octx.enter_context(tc.tile_pool(name="vsb", bufs=1))
        v_sb = vpool.tile([128, NJT, VW], f16, tag="vsb")       # v natural

        # ====== Phase P: q/k/v projections + RoPE, single x stream =========
        with (
            tc.tile_pool(name="wpool", bufs=1) as wpool,
            tc.tile_pool(name="xpool", bufs=3) as xpool,
            tc.tile_pool(name="rope", bufs=2) as rope,
            tc.tile_pool(name="tbl", bufs=1) as tbl,
            tc.tile_pool(name="qk_ps", bufs=2, space="PSUM") as qk_ps,
            tc.tile_pool(name="v_ps", bufs=2, space="PSUM") as v_ps,
        ):
            xc_tiles = {}

            def load_xc(ch):
                t = xpool.tile([128, ES, SC], f16, tag="xc")
                nc.sync.dma_start(
                    t[:],
                    xT[:, ch * SC:(ch + 1) * SC].rearrange(
                        "(es p) s -> p es s", p=128),
                )
                xc_tiles[ch] = t

            load_xc(0)
            wqk = []
            for ct in range(QKT):
                t = wpool.tile([128, ES, 128], f16, tag=f"wqk{ct}")
                nc.sync.dma_start(
                    t[:],
                    Wp[:, ct * 128:(ct + 1) * 128].rearrange(
                        "(es p) c -> p es c", p=128),
                )
                wqk.append(t)
            load_xc(1)
            cos_sb = tbl.tile([128, s], f16, tag="cos")
            nc.sync.dma_start(cos_sb[:], cosP[:, :])
            sin_sb = tbl.tile([128, s], f16, tag="sin")
            nc.sync.dma_start(sin_sb[:], sinP[:, :])
            wv = wpool.tile([128, ES, VW], f16, tag="wv")
            nc.sync.dma_start(
                wv[:], Wp[:, QKT * 128:].rearrange("(es p) c -> p es c", p=128))

            half = HD // 2
            for ch in range(NCH):
                if ch + 2 < NCH:
                    load_xc(ch + 2)
                c0 = ch * SC
                xc = xc_tiles.pop(ch)
                cs = cos_sb[:, c0:c0 + SC]
                sn = sin_sb[:, c0:c0 + SC]

                for pr in range(QKT // 2):
                    pse = qk_ps.tile([128, SC], f32, tag="qkpse")
                    for es in range(ES):
                        nc.tensor.matmul(
                            pse[:], wqk[2 * pr][:, es, :], xc[:, es, :],
                            start=(es == 0), stop=(es == ES - 1),
                        )
                    pso = qk_ps.tile([128, SC], f32, tag="qkpso")
                    for es in range(ES):
                        nc.tensor.matmul(
                            pso[:], wqk[2 * pr + 1][:, es, :], xc[:, es, :],
                            start=(es == 0), stop=(es == ES - 1),
                        )
                    # RoPE straight out of PSUM; partition-shifted writes
                    ta = rope.tile([128, SC], f32, tag="ra")
                    nc.vector.tensor_mul(ta[:], pse[:], cs)
                    tb = rope.tile([128, SC], f32, tag="rb")
                    nc.vector.tensor_mul(tb[:], pso[:], sn)
                    tc2 = rope.tile([128, SC], f32, tag="rc")
                    nc.vector.tensor_mul(tc2[:], pse[:], sn)
                    td = rope.tile([128, SC], f32, tag="rd")
                    nc.vector.tensor_mul(td[:], pso[:], cs)
                    # pair order: q01, q23, k01, k23 (for hpc=4)
                    base = 0 if pr < QKT // 4 else hpc
                    lo = base + 2 * (pr % (QKT // 4))
                    hi = lo + 1
                    # o1 = te*cs - to*sn ; o2 = te*sn + to*cs
                    nc.vector.tensor_sub(
                        qkrot[0:half, lo, c0:c0 + SC], ta[0:half, :], tb[0:half, :])
                    nc.vector.tensor_sub(
                        qkrot[0:half, hi, c0:c0 + SC], ta[half:128, :], tb[half:128, :])
                    nc.vector.tensor_add(
                        qkrot[half:HD, lo, c0:c0 + SC],
                        tc2[0:half, :], td[0:half, :])
                    nc.vector.tensor_add(
                        qkrot[half:HD, hi, c0:c0 + SC],
                        tc2[half:128, :], td[half:128, :])

                for stl in range(SC // 128):
                    st = ch * (SC // 128) + stl
                    ps = v_ps.tile([128, VW], f32, tag="vps")
                    for es in range(ES):
                        nc.tensor.matmul(
                            ps[:], xc[:, es, stl * 128:(stl + 1) * 128],
                            wv[:, es, :],
                            start=(es == 0), stop=(es == ES - 1),
                        )
                    nc.vector.tensor_copy(v_sb[:, st, :], ps[:])

        # ============ Phase A: attention per (head, i-unit) ================
        o2pool = octx.enter_context(tc.tile_pool(name="o2pool", bufs=1))
        out2T = o2pool.tile([128, hpc, s], f32r, tag="out2T")
        wopool = octx.enter_context(tc.tile_pool(name="wopool", bufs=1))
        with (
            tc.tile_pool(name="cpool", bufs=1) as cpool,
            tc.tile_pool(name="expp", bufs=1) as expp,
            tc.tile_pool(name="e8p", bufs=1) as e8p,
            tc.tile_pool(name="rbp", bufs=1) as rbp,
            tc.tile_pool(name="sc_ps", bufs=2, space="PSUM") as sc_ps,
            tc.tile_pool(name="dn_ps", bufs=1, space="PSUM") as dn_ps,
            tc.tile_pool(name="un_ps", bufs=2, space="PSUM") as un_ps,
        ):
            ones16 = cpool.tile([128, 128], f16, tag="ones")
            nc.vector.memset(ones16[:], 1.0)
            wo = wopool.tile([128, hpc, e], f32r, tag="wo")
            nc.sync.dma_start(
                wo[:], WoS.rearrange("(h p) e -> p h e", p=128).bitcast(f32r))

            for h in range(hpc):
                for iu in range(IU):
                    i0 = iu * IHS
                    expt = expp.tile([128, NJT, IHS], f16, tag="expt")
                    e8 = e8p.tile([128, NJT // 2, IHS], f16, tag="e8")
                    for jt in range(NJT):
                        ps = sc_ps.tile([128, IHS], f32, tag="scps")
                        for ic in range(NIC):
                            nc.tensor.matmul(
                                ps[:, ic * NCK:(ic + 1) * NCK],
                                qkrot[:, hpc + h, jt * 128:(jt + 1) * 128],
                                qkrot[:, h, i0 + ic * NCK:i0 + (ic + 1) * NCK],
                            )
                        nc.scalar.activation(expt[:, jt, :], ps[:], Exp, scale=SCALE)
                        if jt % 2 == 1:
                            # incremental level-1 pair add: frees expt early
                            nc.vector.tensor_add(
                                e8[:, jt // 2, :], expt[:, jt - 1, :],
                                expt[:, jt, :])

                    # fold the remaining tree levels in place
                    nt = NJT // 2
                    while nt > 1:
                        nt //= 2
                        nc.vector.tensor_add(
                            e8[:, 0:nt, :], e8[:, 0:nt, :], e8[:, nt:2 * nt, :])
                    dp = dn_ps.tile([128, IHS], f32, tag="dnps")
                    for dc in range(NIC):
                        nc.tensor.matmul(
                            dp[:, dc * NCK:(dc + 1) * NCK], ones16[:, :],
                            e8[:, 0, dc * NCK:(dc + 1) * NCK])
                    rb = rbp.tile([128, IHS], f32, tag="rbc")
                    rsc = rbp.tile([128, IHS], f32, tag="rscratch")
                    nc.vector.reciprocal_approx_accurate(rb[:], dp[:], rsc[:])

                    # out2T = (v.T @ expT) * recip
                    for ic in range(NIC):
                        up = un_ps.tile([128, NCK], f32, tag="unps")
                        for jt in range(NJT):
                            nc.tensor.matmul(
                                up[:], v_sb[:, jt, h * HD:(h + 1) * HD],
                                expt[:, jt, ic * NCK:(ic + 1) * NCK],
                                start=(jt == 0), stop=(jt == NJT - 1),
                            )
                        nc.vector.tensor_mul(
                            out2T[:, h, i0 + ic * NCK:i0 + (ic + 1) * NCK],
                            up[:], rb[:, ic * NCK:(ic + 1) * NCK])

        # ============ Phase O: output projection ===========================
        with (
            tc.tile_pool(name="opool", bufs=2) as opool,
            tc.tile_pool(name="o_ps", bufs=4, space="PSUM") as o_ps,
        ):
            for it in range(s // 128):
                osb = opool.tile([128, e], f32, tag="osb")
                for ec in range(e // 512):
                    op = o_ps.tile([128, 512], f32, tag="ops")
                    for h in range(hpc):
                        nc.tensor.matmul(
                            op[:],
                            out2T[:, h, it * 128:(it + 1) * 128],
                            wo[:, h, ec * 512:(ec + 1) * 512],
                            start=(h == 0), stop=(h == hpc - 1),
                        )
                    nc.scalar.copy(osb[:, ec * 512:(ec + 1) * 512], op[:])
                nc.sync.dma_start(O[it * 128:(it + 1) * 128, :], osb[:])

    nc.compile()    # bacc passes: wait splitting, event sems, library loads
    return nc


# --------------------------------------------------------------------------
# Host-side prep: sharding, transposes, weight permutation, rope tables
# --------------------------------------------------------------------------
def host_prep(x, W_qkv, W_out, s=S, e=E, hpc=HPC, n_heads=H, n_batch=B):
    a = n_heads * HD
    inv = 1.0 / (THETA ** (np.arange(0, HD, 2, dtype=np.float64) / HD))
    fr = np.arange(s, dtype=np.float64)[:, None] * inv[None, :]
    cos = np.cos(fr).T
    sin = np.sin(fr).T
    cosP = np.ascontiguousarray(np.concatenate([cos, cos], axis=0)).astype(np.float16)
    sinP = np.ascontiguousarray(np.concatenate([sin, sin], axis=0)).astype(np.float16)

    cores_per_batch = N_CORES // n_batch
    in_maps = []
    for c in range(N_CORES):
        b = c // cores_per_batch
        h0 = hpc * (c % cores_per_batch)
        heads = [h0 + i for i in range(hpc)]

        xT = np.ascontiguousarray(x[b].T).astype(np.float16)

        cols = []
        for off in (0, a):                           # q block then k block
            for pi in range(hpc // 2):               # head pairs
                pair = heads[2 * pi:2 * pi + 2]
                for par in (0, 1):                   # even tile, odd tile
                    for hh in pair:
                        base = off + hh * HD
                        cols.extend(base + np.arange(par, HD, 2))
        for hh in heads:                             # v natural
            cols.extend(2 * a + hh * HD + np.arange(HD))
        Wp = np.ascontiguousarray(W_qkv[:, np.asarray(cols)]).astype(np.float16)

        rows = np.concatenate([hh * HD + np.arange(HD) for hh in heads])
        WoS = np.ascontiguousarray(W_out[rows])

        in_maps.append({
            "xT": xT, "Wp": Wp, "WoS": WoS, "cosP": cosP, "sinP": sinP,
        })
    return in_maps


@functools.lru_cache(maxsize=1)
def _get_nc():
    return build_bass()


def kernel(x, W_qkv, W_out):
    global LAST_RESULTS
    from concourse import bass_utils

    x = np.ascontiguousarray(np.asarray(x, dtype=np.float32))
    W_qkv = np.ascontiguousarray(np.asarray(W_qkv, dtype=np.float32))
    W_out = np.ascontiguousarray(np.asarray(W_out, dtype=np.float32))

    nc = _get_nc()
    in_maps = host_prep(x, W_qkv, W_out)
    trace = os.environ.get("KERNEL_TRACE", "0") == "1"
    res = bass_utils.run_bass_kernel_spmd(
        nc, in_maps, core_ids=list(range(N_CORES)), trace=trace,
    )
    LAST_RESULTS = res

    cores_per_batch = N_CORES // B
    O = np.zeros((B, S, E), dtype=np.float32)
    for c in range(N_CORES):
        O[c // cores_per_batch] += res.results[c]["O_part"]
    return O

